# revision 35
# baseline (speedup 1.0000x reference)
"""Trainium2 Bass kernel for nn_AffineAdapter (Gaussian blur + affine grid_sample).

The reference pipeline (separable 8-tap Gaussian blur -> bilinear grid_sample on
a 25x25 grid, align_corners=True, zero padding) is linear in x and separable per
axis: each (b, c) image reduces to  out = Ayg^T @ Xg @ Axg  where Xg is the
gathered sub-image formed by the BAND rows/cols under each output sample's
support window and Ayg/Axg (GRID*BAND, GRID) are block-diagonal.

The support per axis is 9 taps, but the outer taps carry ~1e-4..1e-3 of the
mass, so the window is truncated to the best BAND=7 contiguous taps
(runtime-adaptive: widened to 8/9 if the truncated mass exceeds 3e-3 for the
actual sigma/scale).  bf16 throughout lands at rel err ~6e-3 vs the 2e-2
tolerance and shrinks per-image traffic to (25*7)^2 x 2B.

Distribution: pure data parallel over B*C = 128 images -> 16 per core.

Per-core dataflow (tuned against perfetto traces of this axon environment):
  input:    7 concurrent HWDGE DMA queues (each queue only sustains
            ~60-150GB/s; aggregate ~215-300GB/s), 4-image slices ordered so
            wave 0's data lands first.  The B row-chunk rides on 64
            partitions (sub-128-partition DMAs with < 64 partitions
            serialize onto one SDMA engine; 128-partition transfers spread
            over all 16).
  warm-up:  NDUMMY dummy matmuls bridge the PE from program start INTO the
            first real matmul with no idle gap, so the HAM clock gate opens
            (4096-cycle activity window) and everything runs at 2.4GHz.
  stage A:  Z[32g+q, (j, k)] = sum_w Axg[w, q] * XgT[w, (j, k)] per image
            pair (g = pair%4, j = image-in-pair).  The stationary replicates
            Axg into all four 32-col blocks, so one LDWEIGHTS serves every
            pair and each pair's Z lands in its own psum partition group.
            One psum->SBUF bf16 copy per pair (alternating DVE/ACT),
            partition-aligned into zs2[32g:32g+32, u, j, :].
  reshape:  one DVE 32x32 stream-transpose per 8-image half turns
            zs2 [(g,q), u, j, 32kb+ksub] into zt2 [(g,ksub), u, j, kb, q]
            (128 partitions active = 4 blocks per pass).
  stage B:  po[(g,p), (j,q)] += L_kb^T @ zt2[:, u, :, kb, :] over nkb
            k-blocks, with L_kb [128, 100] block-diagonal per g (expanded
            on-device from a compact [32, nkb*25] table).  One copy + one
            output DMA per half (h0's hides under h1's compute).
Semaphore teardown (~8.5us) and preamble are framework-fixed and dominate
the remaining span; instruction/DMA count is kept low to bound them.
"""

import sys

if "/opt/trn_rl_repo" not in sys.path:
    sys.path.insert(0, "/opt/trn_rl_repo")

import numpy as np

GRID = 25
K = 7
KH = K // 2          # conv padding = 3
NTAPS = K + 1        # 8 taps (torch arange quirk)
FULL_BAND = NTAPS + 1  # 9 rows of support per output sample row
H = W = 512
B, C = 16, 8
N_CORES = 8
NIMG = (B * C) // N_CORES  # images per core
HN = NIMG // 2             # images per half
LOST_TOL = 3.0e-3          # max truncated |A| mass per axis
QP = 32                    # q padded to 32 for the DVE block transpose
NDUMMY = 12               # HAM warm-up matmuls (N=512 each, ~0.5us)


def _softplus(v):
    v = np.asarray(v)
    return np.log1p(np.exp(-np.abs(v))) + np.maximum(v, 0.0)


def _axis_weights(lin, g, scale_ax, n_in):
    """(GRID, n_in) float64 weight matrix combining blur taps + bilinear."""
    nb = n_in - 1  # blurred length (conv with K+1 taps, pad K//2 shrinks by 1)
    coord = ((lin * np.float32(scale_ax) + np.float32(1.0))
             * np.float32(0.5) * np.float32(nb - 1)).astype(np.float32)
    c0 = np.floor(coord)
    w1 = (coord - c0).astype(np.float64)
    w0 = 1.0 - w1
    A = np.zeros((GRID, n_in), np.float64)
    g64 = g.astype(np.float64)
    for p in range(GRID):
        for a, wgt in ((0, w0[p]), (1, w1[p])):
            cc = float(c0[p]) + a
            if not (0.0 <= cc <= nb - 1):
                continue  # zero padding_mode: out-of-range corner contributes 0
            ci = int(min(max(cc, 0.0), nb - 1))
            # blurred[ci] = sum_i g[i] * x[ci + i - KH]
            for i in range(NTAPS):
                src = ci + i - KH
                if 0 <= src < n_in:
                    A[p, src] += wgt * g64[i]
    return A


def _build_weights(log_sigma, log_scale):
    # scalar chain in fp32 to mirror the reference
    scale = _softplus(np.asarray(log_scale, np.float32)).astype(np.float32)
    s_min = np.float32(scale.min())
    sigma_min = np.float32(0.0) if s_min >= 1.0 else np.float32(0.44) * (
        np.float32(1.0) / s_min - np.float32(1.0))
    sigma = np.float32(np.sqrt(sigma_min ** 2
                               + _softplus(np.asarray(log_sigma, np.float32)) ** 2))
    taps = np.arange(-(KH + 1), KH + 1, dtype=np.float32)
    g = np.exp(-0.5 * (taps / sigma) ** 2)
    g = g / g.sum()

    lin = np.linspace(-1.0, 1.0, GRID).astype(np.float32)
    Ay = _axis_weights(lin, g, scale[1], H)  # rows scaled by scale[1] (y)
    Ax = _axis_weights(lin, g, scale[0], W)  # cols scaled by scale[0] (x)
    return Ay, Ax


def _best_windows(A, band):
    """Per-sample best contiguous window of width `band`; returns window
    starts r0 and the worst truncated |A| mass."""
    n_in = A.shape[1]
    r0 = np.zeros(GRID, np.int64)
    lost = 0.0
    for p in range(GRID):
        a = np.abs(A[p])
        sup = np.nonzero(a > 0)[0]
        if len(sup) == 0:
            continue
        lo = int(min(max(sup[0] - 1, 0), n_in - band))
        hi = int(min(n_in - band, sup[-1]))
        best, bs = -1.0, lo
        for s in range(lo, hi + 1):
            m = a[s:s + band].sum()
            if m > best:
                best, bs = m, s
        r0[p] = bs
        lost = max(lost, a.sum() - best)
    return r0, lost




def _prepare(log_sigma, log_scale):
    """Choose the band + windows; build the packed stationary operand wts."""
    import ml_dtypes

    Ay, Ax = _build_weights(log_sigma, log_scale)
    for band in (7, 8, 9):
        ry, losty = _best_windows(Ay, band)
        rx, lostx = _best_windows(Ax, band)
        if max(losty, lostx) <= LOST_TOL or band == FULL_BAND:
            break
    ng = GRID * band
    nkb = -(-ng // 32)
    kp = 32 * nkb
    assert 128 < ng <= 256, (band, ng)

    ayg = np.zeros((GRID, band, GRID))
    axg = np.zeros((GRID, band, GRID))
    for p in range(GRID):
        ayg[p, :, p] = Ay[p, ry[p]:ry[p] + band]
        axg[p, :, p] = Ax[p, rx[p]:rx[p] + band]
    ayg = ayg.reshape(ng, GRID)
    axg = axg.reshape(ng, GRID)

    wcols = 256 + GRID * nkb
    wts = np.zeros((128, wcols), np.float64)
    aygp = np.zeros((kp, GRID))
    aygp[:ng] = ayg
    for g in range(4):
        # axgAll/axgBAll: every image-group g gets the same Axg at M-cols 32g+q,
        # so one stationary serves all pairs and psum group-slices stay aligned
        wts[0:128, QP * g:QP * g + GRID] = axg[0:128]
        wts[0:ng - 128, 128 + QP * g:128 + QP * g + GRID] = axg[128:ng]
    # compact Ayg blocks [32, nkb*GRID]; expanded on-device to the
    # block-diagonal stage-B stationaries L_kb [128, 100]
    for kb in range(nkb):
        wts[0:32, 256 + GRID * kb:256 + GRID * (kb + 1)] = \
            aygp[32 * kb:32 * (kb + 1)]
    return band, ry, rx, wts.astype(ml_dtypes.bfloat16)


def _pack_x(x, ry, rx, band):
    """Gather banded rows/cols, cast bf16, transpose to [w, k], split into the
    128-row A chunk and (ng-128)-row B chunk, halves of HN images per core."""
    import ml_dtypes

    ng = GRID * band
    rows = (np.repeat(np.asarray(ry, np.int64), band)
            + np.tile(np.arange(band), GRID))
    cols = (np.repeat(np.asarray(rx, np.int64), band)
            + np.tile(np.arange(band), GRID))
    xf = x.reshape(B * C, H, W)
    xg = xf[:, rows][:, :, cols].astype(ml_dtypes.bfloat16)   # (BC, k, w)
    xgt = np.ascontiguousarray(xg.transpose(0, 2, 1))         # (BC, w, k)
    # (cores, 2 halves, HN, w, k) -> (cores, 2, w, HN, k)
    xh = xgt.reshape(N_CORES, 2, HN, ng, ng).transpose(0, 1, 3, 2, 4)
    nb = ng - 128
    bp = 64 if nb <= 64 else 128
    xa = np.ascontiguousarray(
        xh[:, :, 0:128].transpose(0, 2, 1, 3, 4).reshape(N_CORES, 128, -1))
    # B rows on bp partitions (rows nb:bp zero): [bp, pair, j, k] keeps both
    # images of a pair at partition base 0 so one stationary serves both
    xb = np.zeros((N_CORES, bp, HN, 2, ng), xh.dtype)
    xb[:, 0:nb] = (xh[:, :, 128:ng].transpose(0, 2, 1, 3, 4)
                   .reshape(N_CORES, nb, NIMG, ng)
                   .reshape(N_CORES, nb, HN, 2, ng))
    return xa, xb


_PROGRAM_CACHE = {}


def _build_program(band):
    import concourse.tile as tile
    from concourse import bacc, mybir

    f32 = mybir.dt.float32
    bf16 = mybir.dt.bfloat16
    COPY = mybir.ActivationFunctionType.Copy
    ng = GRID * band
    nb = ng - 128        # rows in chunk B
    bp = 64 if nb <= 64 else 128
    nkb = -(-ng // 32)   # 32-row k blocks
    kp = 32 * nkb
    wcols = 256 + GRID * nkb

    nc = bacc.Bacc("TRN2", target_bir_lowering=False, debug=False,
                   num_devices=N_CORES)
    wtsD = nc.dram_tensor("wts", [128, wcols], bf16, kind="ExternalInput")
    xAD = nc.dram_tensor("xA", [128, NIMG * ng], bf16, kind="ExternalInput")
    xB = nc.dram_tensor("xB", [bp, HN, 2, ng], bf16, kind="ExternalInput")
    out = nc.dram_tensor("out", [4 * GRID, 2, 2, GRID], f32,
                         kind="ExternalOutput")

    with tile.TileContext(nc) as tc:
        with (
            tc.tile_pool(name="const", bufs=1) as cpool,
            tc.tile_pool(name="psd", bufs=1, space="PSUM") as psd_pool,
            tc.tile_pool(name="ps1", bufs=4, space="PSUM") as ps1,
            tc.tile_pool(name="ps2", bufs=2, space="PSUM") as ps2,
        ):
            wtt = cpool.tile([128, wcols], bf16)
            wt = wtt[:]
            xat = cpool.tile([128, NIMG, ng], bf16)
            xta = xat[:]
            xtb = cpool.tile([bp, HN, 2, ng], bf16)
            dum = cpool.tile([128, 512], bf16)
            zs2 = cpool.tile([128, 2, 2, kp], bf16)   # [(g,q), u, j, k]
            zt2 = cpool.tile([128, 2, 2, nkb, QP], bf16)  # [(g,ksub), u, j, kb, q]
            lt = cpool.tile([128, nkb, 4 * GRID], bf16)   # stage-B stationaries
            outst = cpool.tile([4 * GRID, 2, 2, GRID], f32)

            # HWDGE rings sustain ~150GB/s each only with 2800B+ lines and
            # >=2 DMAs in flight; SWDGE (gpsimd) adds a third stream carrying
            # the second half's data
            def xa_slice(a, b):
                return xat[:, a:b, :], xAD[:, a * ng:b * ng].rearrange(
                    "p (i k) -> p i k", i=b - a)
            nc.sync.dma_start(out=wtt[:], in_=wtsD[:])
            nc.scalar.dma_start(out=xtb[:, 0:4], in_=xB[:, 0:4])
            o, i = xa_slice(0, 4)
            nc.sync.dma_start(out=o, in_=i)
            o, i = xa_slice(4, 8)
            nc.scalar.dma_start(out=o, in_=i)
            o, i = xa_slice(8, 12)
            nc.sync.dma_start(out=o, in_=i)
            nc.sync.dma_start(out=xtb[:, 4:HN], in_=xB[:, 4:HN])
            o, i = xa_slice(12, 16)
            nc.scalar.dma_start(out=o, in_=i)

            # HAM warm-up: keep the PE busy through the DMA wait so the clock
            # gate opens (4096-cycle activity window) before real matmuls
            nc.vector.memset(dum[:], 0.0)
            psd = psd_pool.tile([128, 512], f32)
            for _ in range(NDUMMY):
                nc.tensor.matmul(psd[:], dum[:, 0:128], dum[:],
                                 start=True, stop=True)

            nc.vector.memset(zs2[:, :, :, ng:kp], 0.0)

            # expand compact Ayg blocks into block-diagonal L_kb stationaries
            nc.vector.memset(lt[:], 0.0)
            lc = wt[0:32, 256:256 + GRID * nkb].rearrange("p (b q) -> p b q",
                                                          b=nkb)
            for g in range(4):
                nc.vector.tensor_copy(
                    lt[QP * g:QP * (g + 1), :, GRID * g:GRID * (g + 1)], lc)


            def stage_b(u):
                zin = zs2[:, u, :, :].rearrange("p j k -> p (j k)")
                zout = zt2[:, u, :, :, :].rearrange("p j b q -> p (j b q)")
                nc.vector.transpose(zout, zin)
                po = ps2.tile([4 * GRID, 2, QP], f32)
                for kb in range(nkb):
                    nc.tensor.matmul(po[:], lt[:, kb, :],
                                     zt2[:, u, :, kb, :],
                                     start=(kb == 0), stop=(kb == nkb - 1))
                if u == 0:
                    nc.scalar.activation(outst[:, u, :, :], po[:, :, 0:GRID],
                                         COPY)
                    nc.sync.dma_start(out=out[:, 0], in_=outst[:, 0, :, :])
                else:
                    nc.vector.tensor_copy(outst[:, u, :, :], po[:, :, 0:GRID])
                    nc.scalar.dma_start(out=out[:, 1], in_=outst[:, 1, :, :])

            # stage A in two 4-pair waves; waves align with halves (u)
            for wave in range(2):
                pss = []
                for p4 in range(4):
                    pr = wave * 4 + p4
                    ps = ps1.tile([128, 2, ng], f32)
                    nc.tensor.matmul(ps[:], wt[:, 0:128],
                                     xta[:, 2 * pr:2 * pr + 2, :],
                                     start=True, stop=False)
                    pss.append(ps)
                for p4 in range(4):
                    pr = wave * 4 + p4
                    nc.tensor.matmul(pss[p4][:], wt[0:nb, 128:256],
                                     xtb[0:nb, pr, :, :],
                                     start=False, stop=True)
                for p4 in range(4):
                    g = p4
                    dst = zs2[QP * g:QP * (g + 1), wave, :, 0:ng]
                    src = pss[p4][QP * g:QP * (g + 1), :, :]
                    nc.vector.tensor_copy(dst, src)
                stage_b(wave)

    nc.compile()
    return nc


def _get_program(band):
    if band not in _PROGRAM_CACHE:
        _PROGRAM_CACHE[band] = _build_program(band)
    return _PROGRAM_CACHE[band]


def _in_maps(x, band, ry, rx, wts):
    xa, xb = _pack_x(x, ry, rx, band)
    return [
        {"wts": wts, "xA": xa[i], "xB": xb[i]}
        for i in range(N_CORES)
    ]


def _assemble(res):
    out = np.empty((B * C, GRID, GRID), np.float32)
    for i in range(N_CORES):
        # per-core output is [(g, p), u, j, q]; img = 8u + 2g + j
        r = res.results[i]["out"].reshape(4, GRID, 2, 2, GRID)
        out[i * NIMG:(i + 1) * NIMG] = (
            r.transpose(2, 0, 3, 1, 4).reshape(NIMG, GRID, GRID))
    return out.reshape(B, C, GRID, GRID)


def kernel(x, log_sigma, log_scale):
    from concourse.bass_utils import run_bass_kernel_spmd

    x = np.ascontiguousarray(np.asarray(x, np.float32))
    assert x.shape == (B, C, H, W), x.shape

    band, ry, rx, wts = _prepare(log_sigma, log_scale)
    nc = _get_program(band)
    in_maps = _in_maps(x, band, ry, rx, wts)
    res = run_bass_kernel_spmd(nc, in_maps, core_ids=list(range(N_CORES)))
    return _assemble(res)
